# revision 16
# baseline (speedup 1.0000x reference)
"""Causal self-attention (B=4, T=2048, C=1024, H=16) on 8 TRN2 NeuronCores.

Sharding: 8 cores = 4 batches x 2 head-groups (8 heads each). Core c = g*4+b
handles batch b, heads 8g..8g+8. Host transposes x[b] -> xT [C,T] bf16,
slices/arranges W_attn columns (Wq pre-scaled by 1/sqrt(D)) and W_proj rows
per group (all bf16), runs one Bass/Tile kernel SPMD on cores 0-7, sums the
two group-partial out^T [C,T] per batch on host and transposes.

v2 changes vs baseline:
  - N=512 matmul chains in QKV projection and output projection (half the
    PE instruction count).
  - Attention chunks pack the causal spans of both key tiles contiguously
    in PSUM, so each chunk needs exactly ONE activation (exp) and one
    scores matmul per key tile; es is stored packed and PV reads the
    packed offsets (one PV matmul per key tile).
  - Softmax denominators come from PSUM rows 64:127 (v_aug ones columns
    replicate the denominator across 64 partitions) -> vector reciprocal
    + multiply, no gpsimd partition_broadcast.
  - Both diagonal mask strips of a chunk are masked with a single
    strided tensor_mul against a duplicated [128,256] mask.
  - Zero/one fills routed to gpsimd (Pool) to keep DVE free.
"""
import sys
if '/opt/trn_rl_repo' not in sys.path:
    sys.path.insert(0, '/opt/trn_rl_repo')
import numpy as np
import concourse.bacc as bacc
import concourse.tile as tile
import concourse.mybir as mybir
from concourse import bass_utils

F32 = mybir.dt.float32
BF16 = mybir.dt.bfloat16
BF_NP = mybir.dt.np(BF16)

N_EMBED = 1024
N_HEAD = 16
D = 64
B_FULL, T_FULL, C_FULL = 4, 2048, 1024
N_GROUPS = 2
EXPF = mybir.ActivationFunctionType.Exp


def build_kernel(T=T_FULL, C=C_FULL, n_pairs=4, reps=1, mm_w=512):
    HP = n_pairs * 2            # heads per core (8)
    n_k = C // 128              # contraction chunks (8)
    n_jt = T // 128             # key tiles (16)
    n_qt = T // 512             # q tiles (4)
    NS = T // 512               # token strips (4)
    # packed es width for the last q tile (+128 slack: the strided
    # diagonal-mask AP's intermediate slice reaches offs[jt0] + 2*w0)
    ES_W = 2048 * (n_qt - 1) + 1280 + 128

    nc = bacc.Bacc("TRN2", target_bir_lowering=False, debug=False)
    xt_d = nc.dram_tensor("xt", [C, T], BF16, kind="ExternalInput")
    wqk_d = nc.dram_tensor("wqk", [C, n_pairs * 256], BF16, kind="ExternalInput")
    wv_d = nc.dram_tensor("wv", [C, HP * 64], BF16, kind="ExternalInput")
    wp_d = nc.dram_tensor("wp", [HP * 64, C], BF16, kind="ExternalInput")
    mask_d = nc.dram_tensor("mask", [128, 256], BF16, kind="ExternalInput")
    outp_d = nc.dram_tensor("outp", [C, T], BF16, kind="ExternalOutput")

    xt_r = xt_d.ap().rearrange("(k p) t -> p k t", p=128)
    wqk_r = wqk_d.ap().rearrange("(k p) m -> p k m", p=128)
    wv_r = wv_d.ap().rearrange("(k p) m -> p k m", p=128)
    wp_r = wp_d.ap().rearrange("(k p) m -> p k m", p=128)

    with tile.TileContext(nc) as tc:
        with tc.tile_pool(name="wpool", bufs=1) as wpool, \
             tc.tile_pool(name="xpool", bufs=1) as xpool, \
             tc.tile_pool(name="qkpool", bufs=1) as qkpool, \
             tc.tile_pool(name="vpool", bufs=1) as vpool, \
             tc.tile_pool(name="ypool", bufs=1) as ypool, \
             tc.tile_pool(name="espool", bufs=1) as espool, \
             tc.tile_pool(name="npool", bufs=1) as npool, \
             tc.tile_pool(name="ospool", bufs=2) as ospool, \
             tc.tile_pool(name="ps_a", bufs=2, space="PSUM") as ps_a, \
             tc.tile_pool(name="ps_sc", bufs=1, space="PSUM") as ps_sc, \
             tc.tile_pool(name="ps_y", bufs=1, space="PSUM") as ps_y:

            def body(_i=None, staged=False):
                # DMA in consumption order: pair-0 Q/K columns and the
                # first token strip gate the first matmul, so they go
                # first; later pairs' weights arrive during attention.
                wqk_sb = wpool.tile([128, n_k, n_pairs * 256], BF16, tag="wqk",
                                    name="wqk_sb")
                nc.sync.dma_start(out=wqk_sb[:, :, 0:256],
                                  in_=wqk_r[:, :, 0:256])
                xt_sb = xpool.tile([128, n_k, T], BF16, tag="xt", name="xt_sb")
                nc.sync.dma_start(out=xt_sb[:, :, 0:512], in_=xt_r[:, :, 0:512])
                mask_sb = wpool.tile([128, 256], BF16, tag="mask",
                                     name="mask_sb")
                nc.sync.dma_start(out=mask_sb[:], in_=mask_d.ap())
                nc.sync.dma_start(out=xt_sb[:, :, 512:1024],
                                  in_=xt_r[:, :, 512:1024])
                wv_sb = wpool.tile([128, n_k, HP * 64], BF16, tag="wv",
                                   name="wv_sb")
                nc.sync.dma_start(out=wv_sb[:], in_=wv_r)
                for s in range(2, NS):
                    sl = slice(s * 512, (s + 1) * 512)
                    nc.sync.dma_start(out=xt_sb[:, :, sl], in_=xt_r[:, :, sl])
                nc.sync.dma_start(out=wqk_sb[:, :, 256:n_pairs * 256],
                                  in_=wqk_r[:, :, 256:n_pairs * 256])
                wp_sb = wpool.tile([128, HP * 64 // 128, C], BF16, tag="wp",
                                   name="wp_sb")
                nc.sync.dma_start(out=wp_sb[:], in_=wp_r)

                qt = [qkpool.tile([128, T], BF16, tag=f"qt{p}", name=f"qt{p}")
                      for p in range(n_pairs)]
                # K tiles zero-padded per head: kz[h][p] has head h's 64 dims
                # live and the other 64 rows zero, so the scores matmul runs
                # at K=128 with the full qt as rhs.
                kz = [[qkpool.tile([128, T], BF16, tag=f"kz{h}{p}",
                                   name=f"kz{h}{p}")
                       for p in range(n_pairs)] for h in range(2)]
                for p in range(n_pairs):
                    nc.gpsimd.memset(kz[0][p][64:128, :], 0.0)
                    nc.gpsimd.memset(kz[1][p][0:64, :], 0.0)
                # v padded to 128 cols: cols 64+ stay 1.0; PV rows 64:127
                # then hold the softmax denominator replicated 64x.
                v_aug = vpool.tile([128, n_jt, HP, 128], BF16, tag="va",
                                   name="v_aug")
                nc.gpsimd.memset(v_aug[:, :, :, 64:128], 1.0)
                ysb = ypool.tile([128, n_pairs, T], BF16, tag="y", name="ysb")

                def qk_group(p, s, qk, eng):
                    sl = slice(s * 512, (s + 1) * 512)
                    ps = ps_a.tile([128, 512], F32, tag="a", name="qk_ps")
                    col = p * 256 + qk * 128
                    for h0 in range(0, 512, mm_w):
                        for k in range(n_k):
                            nc.tensor.matmul(
                                ps[:, h0:h0 + mm_w],
                                wqk_sb[:, k, col:col + 128],
                                xt_sb[:, k, s * 512 + h0:s * 512 + h0 + mm_w],
                                start=(k == 0), stop=(k == n_k - 1),
                                skip_group_check=True)
                    if qk == 0:
                        eng.tensor_copy(qt[p][:, sl], ps[:])
                    else:
                        eng.tensor_copy(kz[0][p][0:64, sl], ps[0:64, :])
                        eng.tensor_copy(kz[1][p][64:128, sl], ps[64:128, :])

                def v_group(jt, eng):
                    ps = ps_a.tile([128, 512], F32, tag="a", name="v_ps")
                    for h0 in range(0, 512, mm_w):
                        for k in range(n_k):
                            nc.tensor.matmul(
                                ps[:, h0:h0 + mm_w],
                                xt_sb[:, k, jt * 128:(jt + 1) * 128],
                                wv_sb[:, k, h0:h0 + mm_w],
                                start=(k == 0), stop=(k == n_k - 1),
                                skip_group_check=True)
                    for half in range(2):
                        eng.tensor_copy(
                            v_aug[:, jt, half * 4:half * 4 + 4, 0:64],
                            ps[:, half * 256:half * 256 + 256]
                            .rearrange("q (h d) -> q h d", d=D))

                def unit(p, h, qi, slot):
                    """Attention unit (pair, head, q-tile); yields per chunk.

                    Packed layout: key tile jt's live span (width w[jt] =
                    512 - st[jt]) sits at es[:, off[jt]:off[jt]+w[jt]];
                    chunk c = tiles (2c, 2c+1) -> one exp over both spans.
                    """
                    n_kt = 4 * qi + 4
                    hh = p * 2 + h
                    hs = slice(h * 64, (h + 1) * 64)
                    qsl = slice(qi * 512, (qi + 1) * 512)
                    sts = [0] * (4 * qi) + [0, 128, 256, 384]
                    ws = [512 - st for st in sts]
                    offs = [0]
                    for w in ws:
                        offs.append(offs[-1] + w)
                    y_ps = ps_y.tile([128, 512], F32, tag=f"y{slot}",
                                     name=f"y_ps{slot}")
                    es = espool.tile([128, ES_W], BF16, tag=f"es{slot}",
                                     name=f"es{slot}")
                    for c in range(n_kt // 2):
                        jt0, jt1 = 2 * c, 2 * c + 1
                        w0, w1 = ws[jt0], ws[jt1]
                        sc = ps_sc.tile([128, 1024], F32, tag=f"sc{slot}",
                                        name=f"sc{slot}")
                        nc.tensor.matmul(
                            sc[:, 0:w0],
                            kz[h][p][:, jt0 * 128:(jt0 + 1) * 128],
                            qt[p][:, qi * 512 + sts[jt0]:(qi + 1) * 512],
                            start=True, stop=True, skip_group_check=True)
                        nc.tensor.matmul(
                            sc[:, w0:w0 + w1],
                            kz[h][p][:, jt1 * 128:(jt1 + 1) * 128],
                            qt[p][:, qi * 512 + sts[jt1]:(qi + 1) * 512],
                            start=True, stop=True, skip_group_check=True)
                        nc.scalar.activation(
                            out=es[:, offs[jt0]:offs[jt0] + w0 + w1],
                            in_=sc[:, 0:w0 + w1], func=EXPF)
                        if jt0 >= 4 * qi:
                            # both diagonal strips in one strided mul on the
                            # (idle) Pool engine, keeping the exp->PV chain
                            # off the DVE queue: strips at offs[jt0] and
                            # offs[jt0]+w0
                            strips = es[:, offs[jt0]:offs[jt0] + 2 * w0] \
                                .rearrange("q (a b) -> q a b", a=2)[:, :, 0:128]
                            m2 = mask_sb[:].rearrange(
                                "q (a b) -> q a b", a=2)
                            nc.gpsimd.tensor_mul(strips, strips, m2)
                        for jt in (jt0, jt1):
                            nc.tensor.matmul(
                                y_ps[:, sts[jt]:512], v_aug[:, jt, hh, :],
                                es[:, offs[jt]:offs[jt] + ws[jt]],
                                start=(jt == 0), stop=(jt == n_kt - 1),
                                skip_group_check=True)
                        yield
                    recip = npool.tile([64, 512], F32, tag=f"r{slot}",
                                       name=f"recip{slot}")
                    nc.vector.reciprocal(recip[:], y_ps[64:128, :])
                    nc.vector.tensor_mul(ysb[hs, p, qsl], y_ps[0:64, :],
                                         recip[:])

                def ph3_block(s, m, eng):
                    sl = slice(s * 512, (s + 1) * 512)
                    pso = ps_a.tile([128, 512], F32, tag="a", name="pso")
                    for h0 in range(0, 512, mm_w):
                        for kp in range(n_pairs):
                            nc.tensor.matmul(
                                pso[:, h0:h0 + mm_w],
                                wp_sb[:, kp, m * 128:(m + 1) * 128],
                                ysb[:, kp, s * 512 + h0:s * 512 + h0 + mm_w],
                                start=(kp == 0), stop=(kp == n_pairs - 1),
                                skip_group_check=True)
                    osb = ospool.tile([128, 512], BF16, tag="osb",
                                      name="osb")
                    eng.tensor_copy(osb[:], pso[:])
                    nc.sync.dma_start(
                        out=outp_d.ap()[m * 128:(m + 1) * 128, sl],
                        in_=osb[:])

                def ph3_strip(s, eng):
                    for m in range(C // 128):
                        ph3_block(s, m, eng)

                # ---- emission ----
                # phase 1: ALL projections as one dense PE block. A long
                # uninterrupted matmul stream runs at the warm PE clock
                # (~0.46 ns/col measured) vs ~0.7-0.83 when the same
                # groups are interleaved as phase-2 fill between chunks.
                for s in range(NS):
                    for qk in (0, 1):
                        qk_group(0, s, qk, nc.any)
                for jt in range(4):
                    v_group(jt, nc.any)
                for p in range(1, n_pairs):
                    for s in range(NS):
                        for qk in (0, 1):
                            qk_group(p, s, qk, nc.any)
                for jt in range(4, n_jt):
                    v_group(jt, nc.any)

                # phase 2: pure attention pipeline (2 units in flight);
                # PE ~= ACT per chunk, so neither engine idles. Output
                # projection strips still spread into the last pair's
                # rounds (ps_a banks are free of fill work now).
                for p in range(n_pairs):
                    for qi in range(n_qt):
                        g0 = unit(p, 0, qi, 0)
                        g1 = unit(p, 1, qi, 1)
                        if p == n_pairs - 1 and qi >= 1:
                            pend = [(qi - 1, m) for m in range(8)]
                        else:
                            pend = []
                        n_pend = len(pend)
                        rounds = 2 * qi + 2
                        rq = 0
                        done0 = done1 = False
                        while not (done0 and done1):
                            if not done0:
                                done0 = next(g0, "end") == "end"
                            if not done1:
                                done1 = next(g1, "end") == "end"
                            rq += 1
                            while pend and \
                                    len(pend) > n_pend - n_pend * rq // rounds:
                                arg = pend.pop(0)
                                ph3_block(arg[0], arg[1], nc.vector)
                        for arg in pend:
                            ph3_block(arg[0], arg[1], nc.vector)
                ph3_strip(n_qt - 1, nc.vector)

            if reps == 1:
                body()
            else:
                # hint_engines arms the back-edge branch prefetch: the
                # body far exceeds one IRAM block per engine, so without
                # hints every loop iteration pays an I$-miss fetch.
                ET = mybir.EngineType
                with tc.For_i(0, reps, 1,
                              hint_engines=(ET.PE, ET.Activation, ET.DVE,
                                            ET.SP, ET.Pool),
                              staggered_reset=True) as i:
                    body(i, staged=True)
    nc.compile()
    return nc


def host_inputs(x, W_attn, W_proj, n_groups=N_GROUPS):
    """Per-core input maps (bf16). Core order: g * B + b."""
    B, T, C = x.shape
    hp = N_HEAD // n_groups
    n_pairs = hp // 2
    scale = np.float32(1.0 / np.sqrt(D))
    mask1 = (np.arange(128)[None, :] >= np.arange(128)[:, None]).astype(BF_NP)
    mask = np.concatenate([mask1, mask1], axis=1)
    in_maps = []
    for g in range(n_groups):
        qk_cols, v_cols = [], []
        for p in range(n_pairs):
            h0 = g * hp + 2 * p
            qk_cols.append(W_attn[:, h0 * D:(h0 + 2) * D] * scale)   # Q pair
            qk_cols.append(W_attn[:, C + h0 * D:C + (h0 + 2) * D])   # K pair
            v_cols.append(W_attn[:, 2 * C + h0 * D:2 * C + (h0 + 2) * D])
        wqk = np.ascontiguousarray(
            np.concatenate(qk_cols, axis=1)).astype(BF_NP)
        wv = np.ascontiguousarray(np.concatenate(v_cols, axis=1)).astype(BF_NP)
        wp = np.ascontiguousarray(
            W_proj[g * hp * D:(g + 1) * hp * D]).astype(BF_NP)
        for b in range(B):
            xt = np.ascontiguousarray(x[b].T).astype(BF_NP)
            in_maps.append({"xt": xt, "wqk": wqk, "wv": wv, "wp": wp,
                            "mask": mask})
    return in_maps


def host_gather(results, B, T, C, n_groups=N_GROUPS):
    out = np.zeros((B, T, C), dtype=np.float32)
    for g in range(n_groups):
        for b in range(B):
            out[b] += np.asarray(results[g * B + b]["outp"]).astype(np.float32).T
    return out


_NC_CACHE = {}


def kernel(x, W_attn, W_proj):
    x = np.asarray(x, dtype=np.float32)
    W_attn = np.asarray(W_attn, dtype=np.float32)
    W_proj = np.asarray(W_proj, dtype=np.float32)
    B, T, C = x.shape
    if "nc" not in _NC_CACHE:
        _NC_CACHE["nc"] = build_kernel(T=T, C=C)
    nc = _NC_CACHE["nc"]
    in_maps = host_inputs(x, W_attn, W_proj)
    res = bass_utils.run_bass_kernel_spmd(nc, in_maps, core_ids=list(range(8)))
    return host_gather(res.results, B, T, C)


# revision 17
# speedup vs baseline: 1.2841x; 1.2841x over previous
"""Causal self-attention (B=4, T=2048, C=1024, H=16) on 8 TRN2 NeuronCores.

Sharding: 8 cores = 4 batches x 2 head-groups (8 heads each). Core c = g*4+b
handles batch b, heads 8g..8g+8. Host transposes x[b] -> xT [C,T] bf16,
slices/arranges W_attn columns (Wq pre-scaled by 1/sqrt(D)) and W_proj rows
per group (all bf16), runs one Bass/Tile kernel SPMD on cores 0-7, sums the
two group-partial out^T [C,T] per batch on host and transposes.

v2 changes vs baseline:
  - N=512 matmul chains in QKV projection and output projection (half the
    PE instruction count).
  - Attention chunks pack the causal spans of both key tiles contiguously
    in PSUM, so each chunk needs exactly ONE activation (exp) and one
    scores matmul per key tile; es is stored packed and PV reads the
    packed offsets (one PV matmul per key tile).
  - Softmax denominators come from PSUM rows 64:127 (v_aug ones columns
    replicate the denominator across 64 partitions) -> vector reciprocal
    + multiply, no gpsimd partition_broadcast.
  - Both diagonal mask strips of a chunk are masked with a single
    strided tensor_mul against a duplicated [128,256] mask.
  - Zero/one fills routed to gpsimd (Pool) to keep DVE free.
"""
import sys
if '/opt/trn_rl_repo' not in sys.path:
    sys.path.insert(0, '/opt/trn_rl_repo')
import numpy as np
import concourse.bacc as bacc
import concourse.tile as tile
import concourse.mybir as mybir
from concourse import bass_utils

F32 = mybir.dt.float32
BF16 = mybir.dt.bfloat16
BF_NP = mybir.dt.np(BF16)

N_EMBED = 1024
N_HEAD = 16
D = 64
B_FULL, T_FULL, C_FULL = 4, 2048, 1024
N_GROUPS = 2
EXPF = mybir.ActivationFunctionType.Exp


def build_kernel(T=T_FULL, C=C_FULL, n_pairs=4, reps=1, mm_w=512):
    HP = n_pairs * 2            # heads per core (8)
    n_k = C // 128              # contraction chunks (8)
    n_jt = T // 128             # key tiles (16)
    n_qt = T // 512             # q tiles (4)
    NS = T // 512               # token strips (4)
    # packed es width for the last q tile (+128 slack: the strided
    # diagonal-mask AP's intermediate slice reaches offs[jt0] + 2*w0)
    ES_W = 2048 * (n_qt - 1) + 1280 + 128

    nc = bacc.Bacc("TRN2", target_bir_lowering=False, debug=False)
    xt_d = nc.dram_tensor("xt", [C, T], BF16, kind="ExternalInput")
    wqk_d = nc.dram_tensor("wqk", [C, n_pairs * 256], BF16, kind="ExternalInput")
    wv_d = nc.dram_tensor("wv", [C, HP * 64], BF16, kind="ExternalInput")
    wp_d = nc.dram_tensor("wp", [HP * 64, C], BF16, kind="ExternalInput")
    mask_d = nc.dram_tensor("mask", [128, 256], BF16, kind="ExternalInput")
    outp_d = nc.dram_tensor("outp", [C, T], BF16, kind="ExternalOutput")

    xt_r = xt_d.ap().rearrange("(k p) t -> p k t", p=128)
    wqk_r = wqk_d.ap().rearrange("(k p) m -> p k m", p=128)
    wv_r = wv_d.ap().rearrange("(k p) m -> p k m", p=128)
    wp_r = wp_d.ap().rearrange("(k p) m -> p k m", p=128)

    with tile.TileContext(nc) as tc:
        with tc.tile_pool(name="wpool", bufs=1) as wpool, \
             tc.tile_pool(name="xpool", bufs=1) as xpool, \
             tc.tile_pool(name="qkpool", bufs=1) as qkpool, \
             tc.tile_pool(name="vpool", bufs=1) as vpool, \
             tc.tile_pool(name="ypool", bufs=1) as ypool, \
             tc.tile_pool(name="espool", bufs=1) as espool, \
             tc.tile_pool(name="npool", bufs=1) as npool, \
             tc.tile_pool(name="ospool", bufs=2) as ospool, \
             tc.tile_pool(name="ps_a", bufs=2, space="PSUM") as ps_a, \
             tc.tile_pool(name="ps_sc", bufs=1, space="PSUM") as ps_sc, \
             tc.tile_pool(name="ps_y", bufs=1, space="PSUM") as ps_y:

            def body(_i=None, staged=False):
                # DMA in consumption order: pair-0 Q/K columns and the
                # first token strip gate the first matmul, so they go
                # first; later pairs' weights arrive during attention.
                wqk_sb = wpool.tile([128, n_k, n_pairs * 256], BF16, tag="wqk",
                                    name="wqk_sb")
                nc.sync.dma_start(out=wqk_sb[:, :, 0:256],
                                  in_=wqk_r[:, :, 0:256])
                xt_sb = xpool.tile([128, n_k, T], BF16, tag="xt", name="xt_sb")
                nc.sync.dma_start(out=xt_sb[:, :, 0:512], in_=xt_r[:, :, 0:512])
                mask_sb = wpool.tile([128, 256], BF16, tag="mask",
                                     name="mask_sb")
                nc.sync.dma_start(out=mask_sb[:], in_=mask_d.ap())
                nc.sync.dma_start(out=xt_sb[:, :, 512:1024],
                                  in_=xt_r[:, :, 512:1024])
                wv_sb = wpool.tile([128, n_k, HP * 64], BF16, tag="wv",
                                   name="wv_sb")
                nc.sync.dma_start(out=wv_sb[:], in_=wv_r)
                for s in range(2, NS):
                    sl = slice(s * 512, (s + 1) * 512)
                    nc.sync.dma_start(out=xt_sb[:, :, sl], in_=xt_r[:, :, sl])
                nc.sync.dma_start(out=wqk_sb[:, :, 256:n_pairs * 256],
                                  in_=wqk_r[:, :, 256:n_pairs * 256])
                wp_sb = wpool.tile([128, HP * 64 // 128, C], BF16, tag="wp",
                                   name="wp_sb")
                nc.sync.dma_start(out=wp_sb[:], in_=wp_r)

                qt = [qkpool.tile([128, T], BF16, tag=f"qt{p}", name=f"qt{p}")
                      for p in range(n_pairs)]
                # K tiles zero-padded per head: kz[h][p] has head h's 64 dims
                # live and the other 64 rows zero, so the scores matmul runs
                # at K=128 with the full qt as rhs.
                kz = [[qkpool.tile([128, T], BF16, tag=f"kz{h}{p}",
                                   name=f"kz{h}{p}")
                       for p in range(n_pairs)] for h in range(2)]
                for p in range(n_pairs):
                    nc.gpsimd.memset(kz[0][p][64:128, :], 0.0)
                    nc.gpsimd.memset(kz[1][p][0:64, :], 0.0)
                # v padded to 128 cols: cols 64+ stay 1.0; PV rows 64:127
                # then hold the softmax denominator replicated 64x.
                v_aug = vpool.tile([128, n_jt, HP, 128], BF16, tag="va",
                                   name="v_aug")
                nc.gpsimd.memset(v_aug[:, :, :, 64:128], 1.0)
                ysb = ypool.tile([128, n_pairs, T], BF16, tag="y", name="ysb")

                def qk_group(p, s, qk, eng):
                    sl = slice(s * 512, (s + 1) * 512)
                    ps = ps_a.tile([128, 512], F32, tag="a", name="qk_ps")
                    col = p * 256 + qk * 128
                    for h0 in range(0, 512, mm_w):
                        for k in range(n_k):
                            nc.tensor.matmul(
                                ps[:, h0:h0 + mm_w],
                                wqk_sb[:, k, col:col + 128],
                                xt_sb[:, k, s * 512 + h0:s * 512 + h0 + mm_w],
                                start=(k == 0), stop=(k == n_k - 1),
                                skip_group_check=True)
                    if qk == 0:
                        eng.tensor_copy(qt[p][:, sl], ps[:])
                    else:
                        eng.tensor_copy(kz[0][p][0:64, sl], ps[0:64, :])
                        eng.tensor_copy(kz[1][p][64:128, sl], ps[64:128, :])

                def v_group(jt, eng):
                    ps = ps_a.tile([128, 512], F32, tag="a", name="v_ps")
                    for h0 in range(0, 512, mm_w):
                        for k in range(n_k):
                            nc.tensor.matmul(
                                ps[:, h0:h0 + mm_w],
                                xt_sb[:, k, jt * 128:(jt + 1) * 128],
                                wv_sb[:, k, h0:h0 + mm_w],
                                start=(k == 0), stop=(k == n_k - 1),
                                skip_group_check=True)
                    for half in range(2):
                        eng.tensor_copy(
                            v_aug[:, jt, half * 4:half * 4 + 4, 0:64],
                            ps[:, half * 256:half * 256 + 256]
                            .rearrange("q (h d) -> q h d", d=D))

                def unit(p, h, qi, slot):
                    """Attention unit (pair, head, q-tile); yields per chunk.

                    Packed layout: key tile jt's live span (width w[jt] =
                    512 - st[jt]) sits at es[:, off[jt]:off[jt]+w[jt]];
                    chunk c = tiles (2c, 2c+1) -> one exp over both spans.
                    """
                    n_kt = 4 * qi + 4
                    hh = p * 2 + h
                    hs = slice(h * 64, (h + 1) * 64)
                    qsl = slice(qi * 512, (qi + 1) * 512)
                    sts = [0] * (4 * qi) + [0, 128, 256, 384]
                    ws = [512 - st for st in sts]
                    offs = [0]
                    for w in ws:
                        offs.append(offs[-1] + w)
                    y_ps = ps_y.tile([128, 512], F32, tag=f"y{slot}",
                                     name=f"y_ps{slot}")
                    es = espool.tile([128, ES_W], BF16, tag=f"es{slot}",
                                     name=f"es{slot}")
                    for c in range(n_kt // 2):
                        jt0, jt1 = 2 * c, 2 * c + 1
                        w0, w1 = ws[jt0], ws[jt1]
                        sc = ps_sc.tile([128, 1024], F32, tag=f"sc{slot}",
                                        name=f"sc{slot}")
                        nc.tensor.matmul(
                            sc[:, 0:w0],
                            kz[h][p][:, jt0 * 128:(jt0 + 1) * 128],
                            qt[p][:, qi * 512 + sts[jt0]:(qi + 1) * 512],
                            start=True, stop=True, skip_group_check=True)
                        nc.tensor.matmul(
                            sc[:, w0:w0 + w1],
                            kz[h][p][:, jt1 * 128:(jt1 + 1) * 128],
                            qt[p][:, qi * 512 + sts[jt1]:(qi + 1) * 512],
                            start=True, stop=True, skip_group_check=True)
                        nc.scalar.activation(
                            out=es[:, offs[jt0]:offs[jt0] + w0 + w1],
                            in_=sc[:, 0:w0 + w1], func=EXPF)
                        if jt0 >= 4 * qi:
                            # both diagonal strips in one strided mul on the
                            # (idle) Pool engine, keeping the exp->PV chain
                            # off the DVE queue: strips at offs[jt0] and
                            # offs[jt0]+w0
                            strips = es[:, offs[jt0]:offs[jt0] + 2 * w0] \
                                .rearrange("q (a b) -> q a b", a=2)[:, :, 0:128]
                            m2 = mask_sb[:].rearrange(
                                "q (a b) -> q a b", a=2)
                            nc.gpsimd.tensor_mul(strips, strips, m2)
                        for jt in (jt0, jt1):
                            nc.tensor.matmul(
                                y_ps[:, sts[jt]:512], v_aug[:, jt, hh, :],
                                es[:, offs[jt]:offs[jt] + ws[jt]],
                                start=(jt == 0), stop=(jt == n_kt - 1),
                                skip_group_check=True)
                        yield
                    recip = npool.tile([64, 512], F32, tag=f"r{slot}",
                                       name=f"recip{slot}")
                    nc.vector.reciprocal(recip[:], y_ps[64:128, :])
                    nc.vector.tensor_mul(ysb[hs, p, qsl], y_ps[0:64, :],
                                         recip[:])

                def ph3_block(s, m, eng):
                    sl = slice(s * 512, (s + 1) * 512)
                    pso = ps_a.tile([128, 512], F32, tag="a", name="pso")
                    for h0 in range(0, 512, mm_w):
                        for kp in range(n_pairs):
                            nc.tensor.matmul(
                                pso[:, h0:h0 + mm_w],
                                wp_sb[:, kp, m * 128:(m + 1) * 128],
                                ysb[:, kp, s * 512 + h0:s * 512 + h0 + mm_w],
                                start=(kp == 0), stop=(kp == n_pairs - 1),
                                skip_group_check=True)
                    osb = ospool.tile([128, 512], BF16, tag="osb",
                                      name="osb")
                    eng.tensor_copy(osb[:], pso[:])
                    nc.sync.dma_start(
                        out=outp_d.ap()[m * 128:(m + 1) * 128, sl],
                        in_=osb[:])

                def ph3_strip(s, eng):
                    for m in range(C // 128):
                        ph3_block(s, m, eng)

                # ---- emission ----
                # phase 1: pair-0 Q/K + the first 4 V tiles (what unit
                # (p=0, qi=0) needs); remaining V tiles and later pairs'
                # Q/K stream in as fill work between attention chunks.
                for s in range(NS):
                    for qk in (0, 1):
                        qk_group(0, s, qk, nc.any)
                for jt in range(4):
                    v_group(jt, nc.any)

                for p in range(n_pairs):
                    # per-pair fill queue, popped evenly over the pair's
                    # 20 chunk rounds so the PE stream has ready matmuls
                    # at every cross-engine stall point. v_groups lead so
                    # tile jt lands well before the unit that consumes it.
                    fills = []
                    if p == 0:
                        fills += [("v", jt) for jt in range(4, n_jt)]
                    if p < n_pairs - 1:
                        fills += [("qk", (s, qk)) for s in range(NS)
                                  for qk in (0, 1)]
                    n_fill = len(fills)
                    rounds_total = sum(2 * q + 2 for q in range(n_qt))
                    r = 0
                    for qi in range(n_qt):
                        g0 = unit(p, 0, qi, 0)
                        g1 = unit(p, 1, qi, 1)
                        if p == n_pairs - 1 and qi >= 1:
                            pend = [("ph3", (qi - 1, m)) for m in range(8)]
                        else:
                            pend = []
                        n_pend = len(pend)
                        rounds = 2 * qi + 2
                        rq = 0
                        done0 = done1 = False
                        while not (done0 and done1):
                            if not done0:
                                done0 = next(g0, "end") == "end"
                            if not done1:
                                done1 = next(g1, "end") == "end"
                            r += 1
                            rq += 1
                            while fills and len(fills) > \
                                    n_fill - n_fill * r // rounds_total:
                                kind, arg = fills.pop(0)
                                if kind == "qk":
                                    qk_group(p + 1, arg[0], arg[1], nc.vector)
                                else:
                                    v_group(arg, nc.vector)
                            while pend and \
                                    len(pend) > n_pend - n_pend * rq // rounds:
                                _, arg = pend.pop(0)
                                ph3_block(arg[0], arg[1], nc.vector)
                        for _, arg in pend:
                            ph3_block(arg[0], arg[1], nc.vector)
                    for kind, arg in fills:
                        if kind == "qk":
                            qk_group(p + 1, arg[0], arg[1], nc.vector)
                        else:
                            v_group(arg, nc.vector)
                ph3_strip(n_qt - 1, nc.vector)

            if reps == 1:
                body()
            else:
                # hint_engines arms the back-edge branch prefetch: the
                # body far exceeds one IRAM block per engine, so without
                # hints every loop iteration pays an I$-miss fetch.
                ET = mybir.EngineType
                with tc.For_i(0, reps, 1,
                              hint_engines=(ET.PE, ET.Activation, ET.DVE,
                                            ET.SP, ET.Pool),
                              staggered_reset=True) as i:
                    body(i, staged=True)
    nc.compile()
    return nc


def host_inputs(x, W_attn, W_proj, n_groups=N_GROUPS):
    """Per-core input maps (bf16). Core order: g * B + b."""
    B, T, C = x.shape
    hp = N_HEAD // n_groups
    n_pairs = hp // 2
    scale = np.float32(1.0 / np.sqrt(D))
    mask1 = (np.arange(128)[None, :] >= np.arange(128)[:, None]).astype(BF_NP)
    mask = np.concatenate([mask1, mask1], axis=1)
    in_maps = []
    for g in range(n_groups):
        qk_cols, v_cols = [], []
        for p in range(n_pairs):
            h0 = g * hp + 2 * p
            qk_cols.append(W_attn[:, h0 * D:(h0 + 2) * D] * scale)   # Q pair
            qk_cols.append(W_attn[:, C + h0 * D:C + (h0 + 2) * D])   # K pair
            v_cols.append(W_attn[:, 2 * C + h0 * D:2 * C + (h0 + 2) * D])
        wqk = np.ascontiguousarray(
            np.concatenate(qk_cols, axis=1)).astype(BF_NP)
        wv = np.ascontiguousarray(np.concatenate(v_cols, axis=1)).astype(BF_NP)
        wp = np.ascontiguousarray(
            W_proj[g * hp * D:(g + 1) * hp * D]).astype(BF_NP)
        for b in range(B):
            xt = np.ascontiguousarray(x[b].T).astype(BF_NP)
            in_maps.append({"xt": xt, "wqk": wqk, "wv": wv, "wp": wp,
                            "mask": mask})
    return in_maps


def host_gather(results, B, T, C, n_groups=N_GROUPS):
    out = np.zeros((B, T, C), dtype=np.float32)
    for g in range(n_groups):
        for b in range(B):
            out[b] += np.asarray(results[g * B + b]["outp"]).astype(np.float32).T
    return out


_NC_CACHE = {}


def kernel(x, W_attn, W_proj):
    x = np.asarray(x, dtype=np.float32)
    W_attn = np.asarray(W_attn, dtype=np.float32)
    W_proj = np.asarray(W_proj, dtype=np.float32)
    B, T, C = x.shape
    if "nc" not in _NC_CACHE:
        _NC_CACHE["nc"] = build_kernel(T=T, C=C)
    nc = _NC_CACHE["nc"]
    in_maps = host_inputs(x, W_attn, W_proj)
    res = bass_utils.run_bass_kernel_spmd(nc, in_maps, core_ids=list(range(8)))
    return host_gather(res.results, B, T, C)
